# revision 9
# baseline (speedup 1.0000x reference)
"""Trainium2 Bass kernel for the aux-attention module.

reference (per batch b):
    inputs = concat([enc[b], broadcast(hs[b])], -1)          # (S, 4096)
    hidden = tanh(inputs @ W1.T + b1)                        # (S, 1024)
    e      = hidden @ w2.T                                   # (S,)
    alpha  = softmax(e)
    ctx    = alpha @ enc[b]                                  # (3072,)
    out[b] = ctx @ W3.T + b3                                 # (1024,)

Strategy: data-parallel over batch (4 batches/core x 8 cores), weights
replicated. All PE matmuls in fp16 (fp32 PSUM accumulation). Softmax without
max-subtraction: w = exp(e - 4) unnormalized (e is O(1) for this model), the
1/sum(w) normalization is folded into the final output scaling.

Per core:
  pass 1 (per 128-row tile): hiddenT matmul from a host-pretransposed f-major
    copy of enc; tanh on ACT; e-column via one fused DVE multiply+accumulate.
  pass 2 (per row tile): ctx += w_col.T @ enc_nat using the natural layout
    copy of enc, accumulated per 512-wide PSUM bank segment.
  tail: l = per-batch sums of w via two tiny matmuls, ctx transposed 128x4
    on the PE, out = (ctx @ W3.T) * (1/l) + b3.
"""

import numpy as np

import concourse.bass as bass
import concourse.tile as tile
from concourse import mybir
from concourse.bass import ds
from concourse import bass_utils

# ---------------------------------------------------------------------------
# Walrus in this container caps sync waits per instruction; Tile's tail drain
# carries one wait per live semaphore. Split them across a chain of drains.
from concourse import tile as _tile_mod
from concourse import mybir as _mybir


def _patched_drain_and_barrier(self, tick_clock, wait_clock):
    nc = self.nc
    drain_inst = nc.sync.drain()
    wait_clock.add_sem_waits(
        drain_inst.ins, _tile_mod.ScopedClock({None: tick_clock.global_clock})
    )
    si = drain_inst.ins.sync_info
    waits = list(si.on_wait) if si is not None else []
    if len(waits) > 1:
        drain_inst.ins.sync_info = _mybir.SyncInfo(on_update=[], on_wait=waits[:1])
        for w in waits[1:]:
            extra = nc.sync.drain()
            extra.ins.sync_info = _mybir.SyncInfo(on_update=[], on_wait=[w])
    nc.all_engine_barrier()
    assert self.sems is not None
    popped = nc._tile_sem_poison_stack.pop()
    assert popped is self._sem_poison
    nc.clear_and_free_semaphores(list(self.sems.allocated().values()))
    nc.all_engine_barrier()


_tile_mod.TileContext._drain_and_barrier = _patched_drain_and_barrier


def _split_multiwaits(nc):
    """Walrus here accepts at most one sync wait per instruction (two for
    EventSemaphore). Tile occasionally emits more; move extras onto NoOps."""
    for fn in nc.m.functions:
        for blk in fn.blocks:
            out, changed = [], False
            for inst in list(blk.instructions):
                si = inst.sync_info
                waits = list(si.on_wait) if si is not None else []
                cap = 2 if inst.opcode == "EventSemaphore" else 1
                if len(waits) > cap:
                    changed = True
                    for idx, w in enumerate(waits[:-cap]):
                        nop = _mybir.InstNoOp(
                            name=f"{inst.name}-wsplit{idx}", ins=[], outs=[]
                        )
                        nop.engine = inst.engine
                        nop.sync_info = _mybir.SyncInfo(on_update=[], on_wait=[w])
                        out.append(nop)
                    inst.sync_info = _mybir.SyncInfo(
                        on_update=list(si.on_update), on_wait=waits[-cap:]
                    )
                out.append(inst)
            if changed:
                blk.instructions = out


# ---------------------------------------------------------------------------

F16 = mybir.dt.float16
F32 = mybir.dt.float32

N_CORES = 8
B, S, DIM, F = 32, 1024, 1024, 3072  # F = enc feature dim; DIM = model dim
KF = F // 128  # 24 enc k-tiles
KD = DIM // 128  # 8 hs k-tiles / d-blocks
EXP_SHIFT = -4.0  # w = exp(e + EXP_SHIFT); e is O(1), shift keeps fp16 safe


def build_bass(nb, j_tiles):
    """nb batches per core, j_tiles row-tiles of 128 per batch."""
    nj = nb * j_tiles
    nc = bass.Bass()
    encT = nc.declare_dram_parameter("encT", [nj, 128, KF, 128], F16, isOutput=False)
    encN = nc.declare_dram_parameter("encN", [nj, 128, KF, 128], F16, isOutput=False)
    w1t = nc.declare_dram_parameter("w1t", [KF + KD, 128, DIM], F16, isOutput=False)
    w3t = nc.declare_dram_parameter("w3t", [KF, 128, DIM], F16, isOutput=False)
    hst = nc.declare_dram_parameter("hst", [KD, 128, nb], F16, isOutput=False)
    b1r = nc.declare_dram_parameter("b1r", [1, DIM], F16, isOutput=False)
    w2b = nc.declare_dram_parameter("w2b", [128, DIM], F16, isOutput=False)
    b3b = nc.declare_dram_parameter("b3b", [nb, DIM], F32, isOutput=False)
    onesb = nc.declare_dram_parameter("onesb", [128, 128], F16, isOutput=False)
    emat = nc.declare_dram_parameter("emat", [nj, nb], F32, isOutput=False)
    idnb = nc.declare_dram_parameter("idnb", [nb, nb], F16, isOutput=False)
    out_d = nc.declare_dram_parameter("out", [nb, DIM], F32, isOutput=True)

    with tile.TileContext(nc) as tc:
        with (
            tc.tile_pool(name="consts", bufs=1) as consts,
            tc.tile_pool(name="encT", bufs=3) as encT_pool,
            tc.tile_pool(name="encN", bufs=3) as encN_pool,
            tc.tile_pool(name="tanh", bufs=2) as tanh_pool,
            tc.tile_pool(name="scratch", bufs=1) as scratch_pool,
            tc.tile_pool(name="ps", bufs=2, space="PSUM") as ps,
        ):
            # ---- resident constants ----
            w1t_sb = consts.tile([128, KF + KD, DIM], F16)
            for k in range(KF + KD):
                nc.sync.dma_start(out=w1t_sb[:, k, :], in_=w1t[k])
            w3t_sb = consts.tile([128, KF, DIM], F16)
            for k in range(KF):
                nc.sync.dma_start(out=w3t_sb[:, k, :], in_=w3t[k])
            hst_sb = consts.tile([128, KD, nb], F16)
            for k in range(KD):
                nc.sync.dma_start(out=hst_sb[:, k, :], in_=hst[k])
            b1_sb = consts.tile([1, DIM], F16)
            nc.sync.dma_start(out=b1_sb, in_=b1r[:])
            w2b_sb = consts.tile([128, DIM], F16)
            nc.sync.dma_start(out=w2b_sb, in_=w2b[:])
            b3_sb = consts.tile([nb, DIM], F32)
            nc.sync.dma_start(out=b3_sb, in_=b3b[:])
            ones_sb = consts.tile([128, 128], F16)
            nc.sync.dma_start(out=ones_sb, in_=onesb[:])
            emat_sb = consts.tile([nj, nb], F32)
            nc.sync.dma_start(out=emat_sb, in_=emat[:])
            id_sb = consts.tile([nb, nb], F16)
            nc.sync.dma_start(out=id_sb, in_=idnb[:])

            negc_sb = consts.tile([128, 1], F32)
            nc.vector.memset(negc_sb, EXP_SHIFT)

            hb_sb = consts.tile([nb, DIM], F16)
            hbflat_sb = consts.tile([1, nb, DIM], F16)
            e_sb = consts.tile([128, nj], F32)
            w_sb = consts.tile([128, nj], F16)
            ctxrow_sb = consts.tile([1, F], F16)
            ctxall_sb = consts.tile([nb, KF, 128], F16)
            ctxT_sb = consts.tile([128, KF, nb], F16)
            colsums_sb = consts.tile([nj, 1], F32)
            invl_sb = consts.tile([nb, 1], F32)
            out_sb = consts.tile([nb, DIM], F32)

            # ---- hb = hs @ W1h.T + b1 (per-batch bias rows) ----
            for nh in range(2):
                sl = ds(nh * 512, 512)
                hbp = ps.tile([nb, 512], F32, tag="h")
                nc.tensor.matmul(
                    hbp, ones_sb[0:1, 0:nb], b1_sb[0:1, sl], start=True, stop=False
                )
                for k in range(KD):
                    nc.tensor.matmul(
                        hbp,
                        hst_sb[:, k, :],
                        w1t_sb[:, KF + k, sl],
                        start=False,
                        stop=(k == KD - 1),
                    )
                nc.vector.tensor_copy(hb_sb[:, sl], hbp)
            # gather the per-batch bias rows onto partition 0 (matmul rhs
            # operands must start at partition 0)
            nc.sync.dma_start(out=hbflat_sb, in_=hb_sb)

            # ---- main per-batch loop ----
            for b in range(nb):
                # pass 1: hidden = tanh(enc @ W1e.T + hb); e col per row-tile
                for j in range(j_tiles):
                    jj = b * j_tiles + j
                    et = encT_pool.tile([128, KF, 128], F16)
                    nc.sync.dma_start(out=et, in_=encT[jj])
                    th = tanh_pool.tile([128, DIM], F16)
                    for nh in range(2):
                        sl = ds(nh * 512, 512)
                        hp = ps.tile([128, 512], F32, tag="h")
                        nc.tensor.matmul(
                            hp,
                            ones_sb[0:1, :],
                            hbflat_sb[0:1, b, sl],
                            start=True,
                            stop=False,
                        )
                        for k in range(KF):
                            nc.tensor.matmul(
                                hp,
                                et[:, k, :],
                                w1t_sb[:, k, sl],
                                start=False,
                                stop=(k == KF - 1),
                            )
                        nc.scalar.activation(
                            th[:, sl], hp, mybir.ActivationFunctionType.Tanh
                        )
                    sc = scratch_pool.tile([128, DIM], F16)
                    nc.vector.scalar_tensor_tensor(
                        out=sc,
                        in0=th,
                        scalar=1.0,
                        in1=w2b_sb,
                        op0=mybir.AluOpType.mult,
                        op1=mybir.AluOpType.mult,
                        accum_out=e_sb[:, jj : jj + 1],
                    )
                # w = exp(e - 4) for this batch's columns
                nc.scalar.activation(
                    w_sb[:, b * j_tiles : (b + 1) * j_tiles],
                    e_sb[:, b * j_tiles : (b + 1) * j_tiles],
                    mybir.ActivationFunctionType.Exp,
                    bias=negc_sb,
                )
                # pass 2: ctx = sum_s w[s] * enc[s, :]
                cp = ps.tile([1, F], F32, tag="ctx", bufs=1)
                for j in range(j_tiles):
                    jj = b * j_tiles + j
                    en = encN_pool.tile([128, KF, 128], F16)
                    nc.sync.dma_start(out=en, in_=encN[jj])
                    for seg in range(F // 512):
                        nc.tensor.matmul(
                            cp[0:1, ds(seg * 512, 512)],
                            w_sb[:, jj : jj + 1],
                            en[:, 4 * seg : 4 * seg + 4, :],
                            start=(j == 0),
                            stop=(j == j_tiles - 1),
                        )
                nc.vector.tensor_copy(ctxrow_sb, cp)
                nc.sync.dma_start(out=ctxall_sb[b : b + 1, :, :], in_=ctxrow_sb)

            # ---- l = per-batch sums of w; inv_l ----
            csp = ps.tile([nj, 1], F32, tag="h")
            nc.tensor.matmul(csp, w_sb, ones_sb[:, 0:1], start=True, stop=True)
            nc.vector.tensor_copy(colsums_sb, csp)
            lp = ps.tile([nb, 1], F32, tag="h")
            nc.tensor.matmul(lp, emat_sb, colsums_sb, start=True, stop=True)
            nc.vector.reciprocal(invl_sb, lp)

            # ---- ctxT via PE transposes ----
            for c in range(KF):
                tp = ps.tile([128, nb], F16, tag="h")
                nc.tensor.transpose(tp, ctxall_sb[:, c, :], id_sb)
                nc.vector.tensor_copy(ctxT_sb[:, c, :], tp)

            # ---- out = (ctx @ W3.T) * inv_l + b3 ----
            for nh in range(2):
                sl = ds(nh * 512, 512)
                wp = ps.tile([nb, 512], F32, tag="h")
                for k in range(KF):
                    nc.tensor.matmul(
                        wp,
                        ctxT_sb[:, k, :],
                        w3t_sb[:, k, sl],
                        start=(k == 0),
                        stop=(k == KF - 1),
                    )
                nc.vector.scalar_tensor_tensor(
                    out=out_sb[:, sl],
                    in0=wp,
                    scalar=invl_sb,
                    in1=b3_sb[:, sl],
                    op0=mybir.AluOpType.mult,
                    op1=mybir.AluOpType.add,
                )
            nc.sync.dma_start(out=out_d[:], in_=out_sb)

    _split_multiwaits(nc)
    return nc


def make_in_maps(hidden_state, encoder_outputs, W1, b1, w2, W3, b3, nb, j_tiles):
    """Shard + lay out the full inputs for each core. Returns list of dicts."""
    f16, f32 = np.float16, np.float32
    nj = nb * j_tiles
    s_core = j_tiles * 128

    w1t = np.ascontiguousarray(W1.T.reshape(KF + KD, 128, DIM)).astype(f16)
    w3t = np.ascontiguousarray(W3.T.reshape(KF, 128, DIM)).astype(f16)
    b1r = b1.reshape(1, DIM).astype(f16)
    w2b = np.ascontiguousarray(np.broadcast_to(w2.reshape(1, DIM), (128, DIM))).astype(
        f16
    )
    onesb = np.ones((128, 128), f16)
    emat = np.zeros((nj, nb), f32)
    for c in range(nj):
        emat[c, c // j_tiles] = 1.0
    idnb = np.eye(nb, dtype=f16)

    in_maps = []
    for i in range(N_CORES):
        bs = slice(i * nb, (i + 1) * nb)
        enc_c = encoder_outputs[bs, :s_core, :]  # (nb, s_core, F)
        e5 = enc_c.reshape(nb, j_tiles, 128, KF, 128)
        encT = np.ascontiguousarray(e5.transpose(0, 1, 4, 3, 2)).astype(f16)
        encN = np.ascontiguousarray(e5).astype(f16)
        hs_c = hidden_state[bs]  # (nb, DIM)
        hst = np.ascontiguousarray(hs_c.T.reshape(KD, 128, nb)).astype(f16)
        b3b = np.ascontiguousarray(
            np.broadcast_to(b3.reshape(1, DIM), (nb, DIM))
        ).astype(f32)
        in_maps.append(
            {
                "encT": encT.reshape(nj, 128, KF, 128),
                "encN": encN.reshape(nj, 128, KF, 128),
                "w1t": w1t,
                "w3t": w3t,
                "hst": hst,
                "b1r": b1r,
                "w2b": w2b,
                "b3b": b3b,
                "onesb": onesb,
                "emat": emat,
                "idnb": idnb,
            }
        )
    return in_maps


_CACHE = {}


def run(hidden_state, encoder_outputs, W1, b1, w2, W3, b3, nb, j_tiles, trace=False):
    key = (nb, j_tiles)
    if key not in _CACHE:
        _CACHE[key] = build_bass(nb, j_tiles)
    nc = _CACHE[key]
    in_maps = make_in_maps(
        hidden_state, encoder_outputs, W1, b1, w2, W3, b3, nb, j_tiles
    )
    res = bass_utils.run_bass_kernel_spmd(
        nc, in_maps, list(range(N_CORES)), trace=trace
    )
    out = np.concatenate([res.results[i]["out"] for i in range(N_CORES)], axis=0)
    return out.astype(np.float32), res


def kernel(hidden_state, encoder_outputs, W1, b1, w2, W3, b3):
    hidden_state = np.asarray(hidden_state, dtype=np.float32)
    encoder_outputs = np.asarray(encoder_outputs, dtype=np.float32)
    W1 = np.asarray(W1, dtype=np.float32)
    b1 = np.asarray(b1, dtype=np.float32)
    w2 = np.asarray(w2, dtype=np.float32)
    W3 = np.asarray(W3, dtype=np.float32)
    b3 = np.asarray(b3, dtype=np.float32)
    out, _ = run(hidden_state, encoder_outputs, W1, b1, w2, W3, b3, nb=4, j_tiles=8)
    return out


# revision 11
# speedup vs baseline: 1.0968x; 1.0968x over previous
"""Trainium2 Bass kernel for the aux-attention module.

reference (per batch b):
    inputs = concat([enc[b], broadcast(hs[b])], -1)          # (S, 4096)
    hidden = tanh(inputs @ W1.T + b1)                        # (S, 1024)
    e      = hidden @ w2.T                                   # (S,)
    alpha  = softmax(e)
    ctx    = alpha @ enc[b]                                  # (3072,)
    out[b] = ctx @ W3.T + b3                                 # (1024,)

Strategy: data-parallel over batch (4 batches/core x 8 cores), weights
replicated. All PE matmuls in fp16 (fp32 PSUM accumulation). Softmax without
max-subtraction: w = exp(e - 4) unnormalized (e is O(1) for this model), the
1/sum(w) normalization is folded into the final output scaling.

Per core:
  pass 1 (per 128-row tile): hiddenT matmul from a host-pretransposed f-major
    copy of enc; tanh on ACT; e-column via one fused DVE multiply+accumulate.
  pass 2 (per row tile): ctx += w_col.T @ enc_nat using the natural layout
    copy of enc, accumulated per 512-wide PSUM bank segment.
  tail: l = per-batch sums of w via two tiny matmuls, ctx transposed 128x4
    on the PE, out = (ctx @ W3.T) * (1/l) + b3.
"""

import numpy as np

import concourse.bass as bass
import concourse.tile as tile
from concourse import mybir
from concourse.bass import ds
from concourse import bass_utils

# ---------------------------------------------------------------------------
# Walrus in this container caps sync waits per instruction; Tile's tail drain
# carries one wait per live semaphore. Split them across a chain of drains.
from concourse import tile as _tile_mod
from concourse import mybir as _mybir


def _patched_drain_and_barrier(self, tick_clock, wait_clock):
    nc = self.nc
    drain_inst = nc.sync.drain()
    wait_clock.add_sem_waits(
        drain_inst.ins, _tile_mod.ScopedClock({None: tick_clock.global_clock})
    )
    si = drain_inst.ins.sync_info
    waits = list(si.on_wait) if si is not None else []
    if len(waits) > 1:
        drain_inst.ins.sync_info = _mybir.SyncInfo(on_update=[], on_wait=waits[:1])
        for w in waits[1:]:
            extra = nc.sync.drain()
            extra.ins.sync_info = _mybir.SyncInfo(on_update=[], on_wait=[w])
    nc.all_engine_barrier()
    assert self.sems is not None
    popped = nc._tile_sem_poison_stack.pop()
    assert popped is self._sem_poison
    nc.clear_and_free_semaphores(list(self.sems.allocated().values()))
    nc.all_engine_barrier()


_tile_mod.TileContext._drain_and_barrier = _patched_drain_and_barrier


def _split_multiwaits(nc):
    """Walrus here accepts at most one sync wait per instruction (two for
    EventSemaphore). Tile occasionally emits more; move extras onto NoOps."""
    for fn in nc.m.functions:
        for blk in fn.blocks:
            out, changed = [], False
            for inst in list(blk.instructions):
                si = inst.sync_info
                waits = list(si.on_wait) if si is not None else []
                cap = 2 if inst.opcode == "EventSemaphore" else 1
                if len(waits) > cap:
                    changed = True
                    for idx, w in enumerate(waits[:-cap]):
                        nop = _mybir.InstNoOp(
                            name=f"{inst.name}-wsplit{idx}", ins=[], outs=[]
                        )
                        nop.engine = inst.engine
                        nop.sync_info = _mybir.SyncInfo(on_update=[], on_wait=[w])
                        out.append(nop)
                    inst.sync_info = _mybir.SyncInfo(
                        on_update=list(si.on_update), on_wait=waits[-cap:]
                    )
                out.append(inst)
            if changed:
                blk.instructions = out


# ---------------------------------------------------------------------------

F16 = mybir.dt.float16
F32 = mybir.dt.float32

N_CORES = 8
B, S, DIM, F = 32, 1024, 1024, 3072  # F = enc feature dim; DIM = model dim
KF = F // 128  # 24 enc k-tiles
KD = DIM // 128  # 8 hs k-tiles / d-blocks
EXP_SHIFT = -4.0  # w = exp(e + EXP_SHIFT); e is O(1), shift keeps fp16 safe


def build_bass(nb, j_tiles):
    """nb batches per core, j_tiles row-tiles of 128 per batch."""
    nj = nb * j_tiles
    nc = bass.Bass()
    encT = nc.declare_dram_parameter("encT", [nj, 128, KF, 128], F16, isOutput=False)
    encN = nc.declare_dram_parameter("encN", [nj, 128, KF, 128], F16, isOutput=False)
    w1t = nc.declare_dram_parameter("w1t", [KF + KD, 128, DIM], F16, isOutput=False)
    w3t = nc.declare_dram_parameter("w3t", [KF, 128, DIM], F16, isOutput=False)
    hst = nc.declare_dram_parameter("hst", [KD, 128, nb], F16, isOutput=False)
    b1r = nc.declare_dram_parameter("b1r", [1, DIM], F16, isOutput=False)
    w2b = nc.declare_dram_parameter("w2b", [128, DIM], F16, isOutput=False)
    b3b = nc.declare_dram_parameter("b3b", [nb, DIM], F32, isOutput=False)
    onesb = nc.declare_dram_parameter("onesb", [128, 128], F16, isOutput=False)
    emat = nc.declare_dram_parameter("emat", [nj, nb], F32, isOutput=False)
    idnb = nc.declare_dram_parameter("idnb", [nb, nb], F16, isOutput=False)
    out_d = nc.declare_dram_parameter("out", [nb, DIM], F32, isOutput=True)

    with tile.TileContext(nc) as tc:
        with (
            tc.tile_pool(name="consts", bufs=1) as consts,
            tc.tile_pool(name="encT", bufs=3) as encT_pool,
            tc.tile_pool(name="encN", bufs=3) as encN_pool,
            tc.tile_pool(name="tanh", bufs=2) as tanh_pool,
            tc.tile_pool(name="scratch", bufs=1) as scratch_pool,
            tc.tile_pool(name="ps", bufs=2, space="PSUM") as ps,
        ):
            # ---- resident constants ----
            # DMA emission order is the schedule priority: load what the
            # first row-tile needs first so the PE can start within ~10us.
            hst_sb = consts.tile([128, KD, nb], F16)
            for k in range(KD):
                nc.sync.dma_start(out=hst_sb[:, k, :], in_=hst[k])
            b1_sb = consts.tile([1, DIM], F16)
            nc.sync.dma_start(out=b1_sb, in_=b1r[:])
            w2b_sb = consts.tile([128, DIM], F16)
            nc.sync.dma_start(out=w2b_sb, in_=w2b[:])
            ones_sb = consts.tile([128, 128], F16)
            nc.sync.dma_start(out=ones_sb, in_=onesb[:])
            w1t_sb = consts.tile([128, KF + KD, DIM], F16)
            for k in range(KD):  # hs chunks first: the hb stage needs them
                nc.sync.dma_start(out=w1t_sb[:, KF + k, :], in_=w1t[KF + k])
            # first row-tile of enc before the bulk of W1T
            et0 = encT_pool.tile([128, KF, 128], F16, tag="et")
            nc.sync.dma_start(out=et0, in_=encT[0])
            for k in range(KF):
                nc.sync.dma_start(out=w1t_sb[:, k, :], in_=w1t[k])
            # tail-only constants (prefetch whenever DMA is free)
            w3t_sb = consts.tile([128, KF, DIM], F16)
            b3_sb = consts.tile([nb, DIM], F32)
            emat_sb = consts.tile([nj, nb], F32)
            id_sb = consts.tile([nb, nb], F16)

            negc_sb = consts.tile([128, 1], F32)
            nc.vector.memset(negc_sb, EXP_SHIFT)

            hb_sb = consts.tile([nb, DIM], F16)
            hbflat_sb = consts.tile([1, nb, DIM], F16)
            e_sb = consts.tile([128, nj], F32)
            w_sb = consts.tile([128, nj], F16)
            ctxrow_sb = consts.tile([1, F], F16)
            ctxall_sb = consts.tile([nb, KF, 128], F16)
            ctxT_sb = consts.tile([128, KF, nb], F16)
            colsums_sb = consts.tile([nj, 1], F32)
            invl_sb = consts.tile([nb, 1], F32)
            out_sb = consts.tile([nb, DIM], F32)

            # ---- hb = hs @ W1h.T + b1 (per-batch bias rows) ----
            for nh in range(2):
                sl = ds(nh * 512, 512)
                hbp = ps.tile([nb, 512], F32, tag="h")
                for k in range(KD):
                    nc.tensor.matmul(
                        hbp,
                        hst_sb[:, k, :],
                        w1t_sb[:, KF + k, sl],
                        start=(k == 0),
                        stop=False,
                    )
                nc.tensor.matmul(
                    hbp, ones_sb[0:1, 0:nb], b1_sb[0:1, sl], start=False, stop=True
                )
                nc.vector.tensor_copy(hb_sb[:, sl], hbp)
            # gather the per-batch bias rows onto partition 0 (matmul rhs
            # operands must start at partition 0)
            nc.sync.dma_start(out=hbflat_sb, in_=hb_sb)

            # ---- main loop: pass 1 and pass 2 interleaved per row-tile ----
            for b in range(nb):
                cp = ps.tile([1, F], F32, tag="ctx", bufs=1)
                for j in range(j_tiles):
                    jj = b * j_tiles + j
                    if jj == 0:
                        et = et0
                    else:
                        et = encT_pool.tile([128, KF, 128], F16, tag="et")
                        nc.sync.dma_start(out=et, in_=encT[jj])
                    th = tanh_pool.tile([128, DIM], F16)
                    for nh in range(2):
                        sl = ds(nh * 512, 512)
                        hp = ps.tile([128, 512], F32, tag="h")
                        for k in range(KF):
                            nc.tensor.matmul(
                                hp,
                                et[:, k, :],
                                w1t_sb[:, k, sl],
                                start=(k == 0),
                                stop=False,
                            )
                        # bias last so the group doesn't wait on the hb chain
                        nc.tensor.matmul(
                            hp,
                            ones_sb[0:1, :],
                            hbflat_sb[0:1, b, sl],
                            start=False,
                            stop=True,
                        )
                        nc.scalar.activation(
                            th[:, sl], hp, mybir.ActivationFunctionType.Tanh
                        )
                    sc = scratch_pool.tile([128, DIM], F16)
                    nc.vector.scalar_tensor_tensor(
                        out=sc,
                        in0=th,
                        scalar=1.0,
                        in1=w2b_sb,
                        op0=mybir.AluOpType.mult,
                        op1=mybir.AluOpType.mult,
                        accum_out=e_sb[:, jj : jj + 1],
                    )
                    nc.scalar.activation(
                        w_sb[:, jj : jj + 1],
                        e_sb[:, jj : jj + 1],
                        mybir.ActivationFunctionType.Exp,
                        bias=negc_sb,
                    )
                    # pass 2 for this row-tile
                    en = encN_pool.tile([128, KF, 128], F16)
                    nc.sync.dma_start(out=en, in_=encN[jj])
                    for seg in range(F // 512):
                        nc.tensor.matmul(
                            cp[0:1, ds(seg * 512, 512)],
                            w_sb[:, jj : jj + 1],
                            en[:, 4 * seg : 4 * seg + 4, :],
                            start=(j == 0),
                            stop=(j == j_tiles - 1),
                        )
                nc.vector.tensor_copy(ctxrow_sb, cp)
                nc.sync.dma_start(out=ctxall_sb[b : b + 1, :, :], in_=ctxrow_sb)

            # tail-only constant loads (emitted late = low priority)
            for k in range(KF):
                nc.sync.dma_start(out=w3t_sb[:, k, :], in_=w3t[k])
            nc.sync.dma_start(out=b3_sb, in_=b3b[:])
            nc.sync.dma_start(out=emat_sb, in_=emat[:])
            nc.sync.dma_start(out=id_sb, in_=idnb[:])

            # ---- l = per-batch sums of w; inv_l ----
            csp = ps.tile([nj, 1], F32, tag="h")
            nc.tensor.matmul(csp, w_sb, ones_sb[:, 0:1], start=True, stop=True)
            nc.vector.tensor_copy(colsums_sb, csp)
            lp = ps.tile([nb, 1], F32, tag="h")
            nc.tensor.matmul(lp, emat_sb, colsums_sb, start=True, stop=True)
            nc.vector.reciprocal(invl_sb, lp)

            # ---- ctxT via PE transposes ----
            for c in range(KF):
                tp = ps.tile([128, nb], F16, tag="h")
                nc.tensor.transpose(tp, ctxall_sb[:, c, :], id_sb)
                nc.vector.tensor_copy(ctxT_sb[:, c, :], tp)

            # ---- out = (ctx @ W3.T) * inv_l + b3 ----
            for nh in range(2):
                sl = ds(nh * 512, 512)
                wp = ps.tile([nb, 512], F32, tag="h")
                for k in range(KF):
                    nc.tensor.matmul(
                        wp,
                        ctxT_sb[:, k, :],
                        w3t_sb[:, k, sl],
                        start=(k == 0),
                        stop=(k == KF - 1),
                    )
                nc.vector.scalar_tensor_tensor(
                    out=out_sb[:, sl],
                    in0=wp,
                    scalar=invl_sb,
                    in1=b3_sb[:, sl],
                    op0=mybir.AluOpType.mult,
                    op1=mybir.AluOpType.add,
                )
            nc.sync.dma_start(out=out_d[:], in_=out_sb)

    _split_multiwaits(nc)
    return nc


def make_in_maps(hidden_state, encoder_outputs, W1, b1, w2, W3, b3, nb, j_tiles):
    """Shard + lay out the full inputs for each core. Returns list of dicts."""
    f16, f32 = np.float16, np.float32
    nj = nb * j_tiles
    s_core = j_tiles * 128

    w1t = np.ascontiguousarray(W1.T.reshape(KF + KD, 128, DIM)).astype(f16)
    w3t = np.ascontiguousarray(W3.T.reshape(KF, 128, DIM)).astype(f16)
    b1r = b1.reshape(1, DIM).astype(f16)
    w2b = np.ascontiguousarray(np.broadcast_to(w2.reshape(1, DIM), (128, DIM))).astype(
        f16
    )
    onesb = np.ones((128, 128), f16)
    emat = np.zeros((nj, nb), f32)
    for c in range(nj):
        emat[c, c // j_tiles] = 1.0
    idnb = np.eye(nb, dtype=f16)

    in_maps = []
    for i in range(N_CORES):
        bs = slice(i * nb, (i + 1) * nb)
        enc_c = encoder_outputs[bs, :s_core, :]  # (nb, s_core, F)
        e5 = enc_c.reshape(nb, j_tiles, 128, KF, 128)
        encT = np.ascontiguousarray(e5.transpose(0, 1, 4, 3, 2)).astype(f16)
        encN = np.ascontiguousarray(e5).astype(f16)
        hs_c = hidden_state[bs]  # (nb, DIM)
        hst = np.ascontiguousarray(hs_c.T.reshape(KD, 128, nb)).astype(f16)
        b3b = np.ascontiguousarray(
            np.broadcast_to(b3.reshape(1, DIM), (nb, DIM))
        ).astype(f32)
        in_maps.append(
            {
                "encT": encT.reshape(nj, 128, KF, 128),
                "encN": encN.reshape(nj, 128, KF, 128),
                "w1t": w1t,
                "w3t": w3t,
                "hst": hst,
                "b1r": b1r,
                "w2b": w2b,
                "b3b": b3b,
                "onesb": onesb,
                "emat": emat,
                "idnb": idnb,
            }
        )
    return in_maps


_CACHE = {}


def run(hidden_state, encoder_outputs, W1, b1, w2, W3, b3, nb, j_tiles, trace=False):
    key = (nb, j_tiles)
    if key not in _CACHE:
        _CACHE[key] = build_bass(nb, j_tiles)
    nc = _CACHE[key]
    in_maps = make_in_maps(
        hidden_state, encoder_outputs, W1, b1, w2, W3, b3, nb, j_tiles
    )
    res = bass_utils.run_bass_kernel_spmd(
        nc, in_maps, list(range(N_CORES)), trace=trace
    )
    out = np.concatenate([res.results[i]["out"] for i in range(N_CORES)], axis=0)
    return out.astype(np.float32), res


def kernel(hidden_state, encoder_outputs, W1, b1, w2, W3, b3):
    hidden_state = np.asarray(hidden_state, dtype=np.float32)
    encoder_outputs = np.asarray(encoder_outputs, dtype=np.float32)
    W1 = np.asarray(W1, dtype=np.float32)
    b1 = np.asarray(b1, dtype=np.float32)
    w2 = np.asarray(w2, dtype=np.float32)
    W3 = np.asarray(W3, dtype=np.float32)
    b3 = np.asarray(b3, dtype=np.float32)
    out, _ = run(hidden_state, encoder_outputs, W1, b1, w2, W3, b3, nb=4, j_tiles=8)
    return out
